# revision 100
# baseline (speedup 1.0000x reference)
"""Trainium2 Bass kernel for a decoder layer (GQA attention + top-2 MoE FFN).

Sharding over 8 NeuronCores (one SPMD NEFF, per-core input data differs):
  - Attention: core c handles (batch b=c//4, kv-group g=c%4): 4 query heads,
    1 kv head, and the matching out-proj row-slice. Partials are combined
    with a 4-core ReduceScatter (token-sharded); each core adds bias +
    residual for its 128-token shard and computes that shard's router
    logits. The post-attention state is downcast to bf16 (logits ride
    bit-exact as bf16 pairs in columns 1024..1039) and an 8-core
    AllGather (Shared output) gives every core the full [T, 1040] state.
  - MoE: expert-parallel, core c owns expert e=c. Top-2 routing is
    recomputed (replicated, vectorized over all 8 token chunks at once)
    from the shared logits; ranks come from one triangular-matmul cumsum
    plus a chunk-offset fixup. Each core scatters only 8-byte
    (weight, tokid) records into a compact table, indirect-GATHERS its
    expert's <=C_CAP token rows (bf16) straight from the AllGather
    buffer, RMS-normalizes them, runs the dense FFN (bf16 weights, w2
    preloaded in SBUF so block outputs scatter while the next block
    computes), scatters weighted bf16 outputs back to token rows of a
    zeroed [T, D] partial buffer, and an 8-core ReduceScatter sums the
    expert contributions. Each core emits its 128-token output shard;
    the host concatenates shards into the full [B, S, D] output.
"""
import os

import ml_dtypes
import numpy as np

import concourse.bass as bass
import concourse.mybir as mybir
import concourse.tile as tile
from concourse import bacc
from concourse import bass_utils
from concourse.masks import make_identity

# model dims (hardcoded per problem spec)
B, S, D = 2, 512, 1024
H, KV, HD = 16, 4, 64
E, FF, TOPK = 8, 4096, 2
EPS = 1e-6
T = B * S          # 1024 tokens
P = 128
NCORES = 8
C_CAP = 320        # per-expert token capacity (mean 256, +4.6 sigma)
CB = 3             # capacity blocks (ragged: 128 + 128 + 64)
CBS = [(0, P), (P, P), (2 * P, C_CAP - 2 * P)]  # (start, size) per block
DCH = D // P       # 8
FFCH = FF // P     # 32
TCH = T // P       # 8
SB = S // P        # 4
GWB = D + 16       # bf16 AG row: 1024 data + 8 f32 logits as 16 bf16

F32 = mybir.dt.float32
BF16 = mybir.dt.bfloat16
I32 = mybir.dt.int32
AF = mybir.ActivationFunctionType
ALU = mybir.AluOpType
AXL = mybir.AxisListType


def _mm(nc, out, lhsT, rhs, start, stop, dt=None):
    if dt is not None and dt != F32:
        if lhsT.dtype != dt:
            lhsT = lhsT.bitcast(dt)
        if rhs.dtype != dt:
            rhs = rhs.bitcast(dt)
    nc.tensor.matmul(out, lhsT=lhsT, rhs=rhs, start=start, stop=stop)


def build(nc: bass.Bass):
    def dram(n, s, d=F32):
        return nc.dram_tensor(n, s, d, kind="ExternalInput")

    tn = {}
    tn["xb"] = dram("xb", [S, D], BF16)      # x[b] for this core's batch
    tn["xpb"] = dram("xpb", [P, D])          # (x + bo) rows [c*128:(c+1)*128]
    tn["cosT"] = dram("cosT", [P, S], BF16)  # rope cos^T duplicated rows
    tn["sinT"] = dram("sinT", [P, S], BF16)
    tn["rotm"] = dram("rotm", [P, P], BF16)  # rot_half as matmul lhsT
    tn["wq"] = dram("wq", [D, 4 * HD], BF16)  # this core's 4 query heads
    tn["wk"] = dram("wk", [D, 2 * HD], BF16)  # kv head dup to both halves
    tn["wv"] = dram("wv", [D, HD], BF16)
    tn["bq"] = dram("bq", [P, 2])
    tn["bk"] = dram("bk", [2 * HD, 1])
    tn["bv"] = dram("bv", [1, HD])
    tn["wo"] = dram("wo", [4 * HD, D], BF16)  # rows g*256..(g+1)*256 of wo
    tn["rw"] = dram("rw", [P, DCH * E])      # (router_w*norm2_w) packed
    tn["rb"] = dram("rb", [1, E])
    tn["xpbrw"] = dram("xpbrw", [P, E])      # (x+bo) shard @ rw, host-side
    tn["mtri"] = dram("mtri", [P, P])        # additive causal mask (0/-1e5)
    tn["w1"] = dram("w1", [FFCH, P, D], BF16)  # w1h[mf,p,kd*128+f]
    tn["b1T"] = dram("b1T", [P, FFCH])
    tn["w2"] = dram("w2", [FF, D], BF16)
    tn["b2"] = dram("b2", [1, D])
    tn["tokid"] = dram("tokid", [P, TCH])    # tc*128+p as f32
    tn["iotac"] = dram("iotac", [1, C_CAP])  # 0..C_CAP-1 as f32
    tn["esel"] = dram("esel", [1, E])        # one-hot row for expert e
    tn["out_sh"] = nc.dram_tensor("out_sh", [P, D], F32, kind="ExternalOutput")
    if os.environ.get("KDBG") == "1":
        out = lambda n, s, d=F32: nc.dram_tensor(n, s, d, kind="ExternalOutput")
        tn["dbg_xs"] = out("dbg_xs", [P, D])
        tn["dbg_sel"] = out("dbg_sel", [P, TCH])
        tn["dbg_h2"] = out("dbg_h2", [P, CB * D], BF16)
        tn["dbg_hT"] = out("dbg_hT", [P, C_CAP], BF16)

    with tile.TileContext(nc) as tc:
        _build_tc(nc, tc, tn)
    return nc


def _build_tc(nc, tc, tn):
    with (
        tc.tile_pool(name="consts", bufs=1) as consts,
        tc.tile_pool(name="persist", bufs=1) as persist,
        tc.tile_pool(name="dram", bufs=1, space="DRAM") as dpool,
    ):
        ident = consts.tile([P, P], F32)
        make_identity(nc, ident[:])
        identb = consts.tile([P, P], BF16)
        make_identity(nc, identb[:])

        # w2 preload buffer (DMA issued after attention weight loads)
        w2sb = persist.tile([P, FFCH, D], BF16)

        # ---- DRAM buffers (zero/init DMAs issued later, in phase B) ----
        zerob = consts.tile([P, D], BF16)
        nc.vector.memset(zerob[:], 0.0)
        partial_d = dpool.tile([T + P, D], BF16)    # rows T.. = trash
        po_d = dpool.tile([S, D], BF16)
        rs_att = dpool.tile([P, D], BF16)
        # AG rows: 1024 normalized-h2 + 8 top-2 sel + 8 weight-delta riders
        xs_d = dpool.tile([P, GWB], BF16)
        xatt_d = dpool.tile([T, GWB], BF16, addr_space="Shared")
        moe_sh = dpool.tile([P, D], BF16)

        # long-lived SBUF
        xs_t = persist.tile([P, D], F32)            # shard residual state
        h2gT = persist.tile([P, DCH, C_CAP], BF16)  # compacted tokens (d-maj)
        wg_t = persist.tile([P, CB], F32)
        id_i = persist.tile([P, CB], I32)

        # =================== phase A: attention ===================
        with (
            tc.tile_pool(name="pa", bufs=1) as pa,
            tc.tile_pool(name="wa", bufs=3) as wa,
            tc.tile_pool(name="was", bufs=3) as was,
            tc.tile_pool(name="ps512", bufs=3, space="PSUM") as ps512,
            tc.tile_pool(name="pstp", bufs=2, space="PSUM") as pstp,
            tc.tile_pool(name="pssm", bufs=3, space="PSUM") as pssm,
        ):
            def transpose_to(dst_ap, src_ap, idn, cp=None):
                dt = idn.dtype
                pt = pstp.tile([P, P], dt,
                               tag="tpf" if dt == F32 else "tpb")
                nc.tensor.transpose(pt[:], src_ap, idn)
                (cp or nc.scalar.copy)(dst_ap, pt[:])

            x_t = pa.tile([P, SB, D], BF16)
            nc.sync.dma_start(x_t[:],
                              tn["xb"][:].rearrange("(o p) d -> p o d", p=P))

            # rms norm 1 -> h1 (token layout)
            h1_t = pa.tile([P, SB, D], BF16)
            for tb in range(SB):
                sq = wa.tile([P, D], F32, tag="sq")
                ssq = was.tile([P, 1], F32, tag="ssq")
                nc.scalar.activation(sq[:], x_t[:, tb], AF.Square,
                                     accum_out=ssq[:])
                ms = was.tile([P, 1], F32, tag="ms")
                nc.vector.tensor_scalar(ms[:], ssq[:], 1.0 / D, EPS,
                                        ALU.mult, ALU.add)
                rinv = was.tile([P, 1], F32, tag="rinv")
                nc.vector.reciprocal(rinv[:], ms[:])
                rsq = was.tile([P, 1], F32, tag="rsq")
                nc.scalar.sqrt(rsq[:], rinv[:])
                nc.vector.tensor_scalar_mul(h1_t[:, tb], x_t[:, tb], rsq[:])

            # transpose h1 -> h1T [p=d, dc, tok]
            h1T = pa.tile([P, DCH, S], BF16)
            for tb in range(SB):
                for dc in range(DCH):
                    transpose_to(h1T[:, dc, tb * P:(tb + 1) * P],
                                 h1_t[:, tb, dc * P:(dc + 1) * P], identb[:],
                                 cp=nc.vector.tensor_copy)

            # q projection -> qT [p, m, tok]
            wq_t = pa.tile([P, DCH, 4 * HD], BF16)
            nc.sync.dma_start(wq_t[:],
                              tn["wq"][:].rearrange("(o p) n -> p o n", p=P))
            bq_t = pa.tile([P, 2], F32)
            nc.sync.dma_start(bq_t[:], tn["bq"][:])
            qT = pa.tile([P, 2, S], BF16)
            for m in range(2):
                pt = ps512.tile([P, 512], F32, tag="mm512")
                for kd in range(DCH):
                    _mm(nc, pt[:], wq_t[:, kd, m * P:(m + 1) * P], h1T[:, kd],
                        kd == 0, kd == DCH - 1)
                nc.scalar.activation(qT[:, m], pt[:], AF.Identity,
                                     bias=bq_t[:, m:m + 1])

            # k projection (kv head duplicated to both halves) -> kT [128, S]
            wk_t = pa.tile([P, DCH, 2 * HD], BF16)
            nc.sync.dma_start(wk_t[:],
                              tn["wk"][:].rearrange("(o p) n -> p o n", p=P))
            bk_t = pa.tile([2 * HD, 1], F32)
            nc.sync.dma_start(bk_t[:], tn["bk"][:])
            kT = pa.tile([P, S], BF16)
            ptk = ps512.tile([P, 512], F32, tag="mm512")
            for kd in range(DCH):
                _mm(nc, ptk[:], wk_t[:, kd], h1T[:, kd], kd == 0,
                    kd == DCH - 1)
            nc.scalar.activation(kT[:], ptk[:], AF.Identity,
                                 bias=bk_t[:, 0:1])

            # v projection -> v_t [p=tok, tb, 64] (token layout)
            wv_t = pa.tile([P, DCH, HD], BF16)
            nc.sync.dma_start(wv_t[:],
                              tn["wv"][:].rearrange("(o p) n -> p o n", p=P))
            bv_t = pa.tile([P, HD], F32)
            nc.sync.dma_start(bv_t[:], tn["bv"][:].to_broadcast((P, HD)))
            v_t = pa.tile([P, SB, HD], BF16)
            for tb in range(SB):
                pt = pssm.tile([P, HD], F32, tag="sm")
                for kd in range(DCH):
                    _mm(nc, pt[:], h1T[:, kd, tb * P:(tb + 1) * P],
                        wv_t[:, kd], kd == 0, kd == DCH - 1)
                nc.vector.tensor_tensor(v_t[:, tb], pt[:], bv_t[:],
                                        ALU.add)

            # rope: rot_half via rotation-matrix matmul (no partition shifts)
            cos_t = consts.tile([P, S], BF16)
            sin_t = consts.tile([P, S], BF16)
            nc.sync.dma_start(cos_t[:], tn["cosT"][:])
            nc.sync.dma_start(sin_t[:], tn["sinT"][:])
            rotm_t = consts.tile([P, P], BF16)
            nc.sync.dma_start(rotm_t[:], tn["rotm"][:])

            def rope(dst):  # dst: [128, S] AP (two 64-d groups), in place
                ptr_ = ps512.tile([P, S], F32, tag="mm512")
                _mm(nc, ptr_[:], rotm_t[:], dst, True, True)
                t1 = wa.tile([P, S], BF16, tag="ropet1")
                nc.vector.tensor_tensor(t1[:], dst, cos_t[:], ALU.mult)
                t2 = wa.tile([P, S], BF16, tag="ropet2")
                nc.vector.tensor_tensor(t2[:], ptr_[:], sin_t[:], ALU.mult)
                nc.vector.tensor_tensor(dst, t1[:], t2[:], ALU.add)

            for m in range(2):
                rope(qT[:, m])
            rope(kT[:])

            # scores -> softmax -> AV per head / query block
            mtri_t = consts.tile([P, P], F32)
            nc.sync.dma_start(mtri_t[:], tn["mtri"][:])
            # w2 preload: issued after every attention-critical DMA so it
            # fills the DMA engines only during compute / collectives
            nc.sync.dma_start(w2sb[:],
                              tn["w2"][:].rearrange("(o p) d -> p o d", p=P))
            o_t = pa.tile([P, SB, 4 * HD], BF16)
            oT = pa.tile([P, 2, S], BF16)
            wo_t = pa.tile([P, 2, D], BF16)
            nc.sync.dma_start(wo_t[:],
                              tn["wo"][:].rearrange("(o p) n -> p o n", p=P))
            for i in range(SB):
                for h in range(4):
                    nk = (i + 1) * P
                    hb = (h % 2) * HD
                    q_ap = qT[hb:hb + HD, h // 2, i * P:(i + 1) * P]
                    ps_s = ps512.tile([P, 512], F32, tag="mm512")
                    _mm(nc, ps_s[:, :nk], q_ap, kT[hb:hb + HD, :nk], True,
                        True)
                    nc.vector.tensor_tensor(ps_s[:, i * P:nk],
                                            ps_s[:, i * P:nk],
                                            mtri_t[:], ALU.add)
                    nm = was.tile([P, 1], F32, tag="negmax")
                    nc.vector.tensor_reduce(nm[:], ps_s[:, :nk], AXL.X,
                                            ALU.max, negate=True)
                    pr = wa.tile([P, 512], BF16, tag="probs")
                    ssum = was.tile([P, 1], F32, tag="ssum")
                    nc.scalar.activation(pr[:, :nk], ps_s[:, :nk], AF.Exp,
                                         bias=nm[:], accum_out=ssum[:])
                    rs = was.tile([P, 1], F32, tag="rsum")
                    nc.vector.reciprocal(rs[:], ssum[:])
                    ps_o = pssm.tile([P, HD], F32, tag="sm")
                    for j in range(i + 1):
                        pT = wa.tile([P, P], BF16, tag="pT")
                        transpose_to(pT[:], pr[:, j * P:(j + 1) * P],
                                     identb[:], cp=nc.vector.tensor_copy)
                        _mm(nc, ps_o[:], pT[:], v_t[:, j], j == 0, j == i)
                    nc.vector.tensor_scalar_mul(
                        o_t[:, i, h * HD:(h + 1) * HD], ps_o[:], rs[:])

                # token block i complete: transpose + out-projection + DMA
                tb = i
                for m in range(2):
                    transpose_to(oT[:, m, tb * P:(tb + 1) * P],
                                 o_t[:, tb, m * P:(m + 1) * P], identb[:])
                for nh in range(2):
                    pt = ps512.tile([P, 512], F32, tag="mm512")
                    for ko in range(2):
                        _mm(nc, pt[:], oT[:, ko, tb * P:(tb + 1) * P],
                            wo_t[:, ko, nh * 512:(nh + 1) * 512],
                            ko == 0, ko == 1)
                    po_sb = wa.tile([P, 512], BF16, tag="posb")
                    nc.vector.tensor_copy(po_sb[:], pt[:])
                    nc.sync.dma_start(
                        po_d[tb * P:(tb + 1) * P, nh * 512:(nh + 1) * 512],
                        po_sb[:])

            # prime the Act sqrt function table right after the last softmax
            # exp (real sqrt below then skips the ~1.3us table swap); the
            # result feeds the logit chain with zero weight so it is neither
            # dead code nor hoistable before the exps
            dumm = was.tile([P, 1], F32, tag="dumm")
            nc.scalar.sqrt(dumm[:], ssum[:])
            zflag = was.tile([P, 1], F32, tag="zflag")
            nc.vector.tensor_scalar(zflag[:], dumm[:], 0.0, None, ALU.mult)

            # 4-core ReduceScatter within batch group -> 128-token shard
            nc.gpsimd.collective_compute(
                "ReduceScatter", ALU.add,
                replica_groups=[[0, 1, 2, 3], [4, 5, 6, 7]],
                ins=[po_d[:].opt()], outs=[rs_att[:].opt()])

            # shard: add residual + bo; compute shard router logits; pack.
            # logits = rsq * (rsb@rwn + xpb@rwn) + rb with xpb@rwn host-side
            rsb = wa.tile([P, D], BF16, tag="rsb")
            nc.sync.dma_start(rsb[:], rs_att[:])
            xpb_t = wa.tile([P, D], F32, tag="probs2")
            nc.sync.dma_start(xpb_t[:], tn["xpb"][:])
            nc.vector.tensor_tensor(xs_t[:], rsb[:], xpb_t[:], ALU.add)

            rsT = pa.tile([P, DCH, P], F32)   # bf16 values lifted exactly
            for dc in range(DCH):
                transpose_to(rsT[:, dc], rsb[:, dc * P:(dc + 1) * P],
                             identb[:])
            sq = wa.tile([P, D], F32, tag="sq")
            ssq = was.tile([P, 1], F32, tag="ssq")
            nc.scalar.activation(sq[:], xs_t[:], AF.Square,
                                 accum_out=ssq[:])
            ms = was.tile([P, 1], F32, tag="ms")
            nc.vector.tensor_scalar(ms[:], ssq[:], 1.0 / D, EPS, ALU.mult,
                                    ALU.add)
            rinv = was.tile([P, 1], F32, tag="rinv")
            nc.vector.reciprocal(rinv[:], ms[:])
            rsq = was.tile([P, 1], F32, tag="rsq")
            nc.scalar.sqrt(rsq[:], rinv[:])
            rw_t = consts.tile([P, DCH, E], F32)
            nc.sync.dma_start(rw_t[:], tn["rw"][:].rearrange(
                "p (o n) -> p o n", n=E))
            rb_t = consts.tile([P, E], F32)
            nc.sync.dma_start(rb_t[:], tn["rb"][:].to_broadcast((P, E)))
            xpbrw_t = consts.tile([P, E], F32)
            nc.sync.dma_start(xpbrw_t[:], tn["xpbrw"][:])
            ptl = pssm.tile([P, HD], F32, tag="sm")
            for dc in range(DCH):
                _mm(nc, ptl[:, :E], rsT[:, dc], rw_t[:, dc], dc == 0,
                    dc == DCH - 1)
            lg = was.tile([P, E], F32, tag="lg")
            nc.vector.tensor_tensor(lg[:], ptl[:, :E], xpbrw_t[:], ALU.add)
            nc.vector.tensor_scalar(lg[:], lg[:], rsq[:], zflag[:],
                                    ALU.mult, ALU.add)
            nc.vector.tensor_tensor(lg[:], lg[:], rb_t[:], ALU.add)
            # shard-local top-2 (monotone in logits, no exp needed) and the
            # weight delta: w_e = sigmoid(2*lg_e - v1 - v2), applied post-AG
            v1n = was.tile([P, 1], F32, tag="v1n")
            nc.vector.tensor_reduce(v1n[:], lg[:], AXL.X, ALU.max)
            s1 = was.tile([P, E], F32, tag="s1a")
            nc.vector.tensor_scalar(s1[:], lg[:], v1n[:], None, ALU.is_equal)
            e2m = was.tile([P, E], F32, tag="e2m")
            nc.vector.tensor_scalar(e2m[:], s1[:], -1e9, None, ALU.mult)
            nc.vector.tensor_tensor(e2m[:], lg[:], e2m[:], ALU.add)
            v2n = was.tile([P, 1], F32, tag="v2n")
            nc.vector.tensor_reduce(v2n[:], e2m[:], AXL.X, ALU.max)
            s2 = was.tile([P, E], F32, tag="s2a")
            nc.vector.tensor_scalar(s2[:], e2m[:], v2n[:], None,
                                    ALU.is_equal)
            selr = was.tile([P, E], F32, tag="selr")
            nc.vector.tensor_tensor(selr[:], s1[:], s2[:], ALU.add)
            vs = was.tile([P, 1], F32, tag="vs")
            nc.vector.tensor_tensor(vs[:], v1n[:], v2n[:], ALU.add)
            dl = was.tile([P, E], F32, tag="dl")
            nc.vector.tensor_scalar(dl[:], lg[:], 2.0, None, ALU.mult)
            nc.vector.tensor_scalar(dl[:], dl[:], vs[:], None, ALU.subtract)
            # normalized h2 shard (norm2_w folded into w1 host-side)
            xsb = pa.tile([P, GWB], BF16)
            nc.vector.tensor_scalar_mul(xsb[:, :D], xs_t[:], rsq[:])
            nc.vector.tensor_copy(xsb[:, D:D + E], selr[:])
            nc.vector.tensor_copy(xsb[:, D + E:D + 2 * E], dl[:])
            nc.sync.dma_start(xs_d[:], xsb[:])
            if "dbg_xs" in tn:
                nc.sync.dma_start(tn["dbg_xs"][:], xs_t[:])

        # 8-core AllGather: normalized h2 + routing riders (bf16 rows)
        nc.gpsimd.collective_compute(
            "AllGather", ALU.bypass,
            replica_groups=[[0, 1, 2, 3, 4, 5, 6, 7]],
            ins=[xs_d[:].opt()], outs=[xatt_d[:].opt()])

        # =================== phase B: routing + dispatch ===================
        with (
            tc.tile_pool(name="pb", bufs=1) as pb,
            tc.tile_pool(name="wb", bufs=2) as wb,
            tc.tile_pool(name="wbs", bufs=3) as wbs,
            tc.tile_pool(name="psb", bufs=1, space="PSUM") as psb,
            tc.tile_pool(name="psbt", bufs=2, space="PSUM") as psbt,
        ):
            # routing riders for all tokens: [p, chunk, 2E] bf16
            rid = pb.tile([P, TCH, 2 * E], BF16)
            nc.sync.dma_start(
                rid[:],
                xatt_d[:, D:D + 2 * E].rearrange("(o p) d -> p o d", p=P))
            tokid_t = consts.tile([P, TCH], F32)
            nc.sync.dma_start(tokid_t[:], tn["tokid"][:])
            iota_t = consts.tile([P, C_CAP], F32)
            nc.sync.dma_start(iota_t[:], tn["iotac"][:].to_broadcast(
                (P, C_CAP)))
            # deferred DRAM zeroing: needed by the FFN2 scatters
            for i_ in range(TCH):
                nc.sync.dma_start(partial_d[i_ * P:(i_ + 1) * P, :],
                                  zerob[:])
            esel_t = consts.tile([P, E], F32)
            nc.sync.dma_start(esel_t[:], tn["esel"][:].to_broadcast((P, E)))
            ustrict = consts.tile([P, P], F32)
            nc.vector.memset(ustrict[:], 1.0)
            # keep 1.0 where p < f (iota = f - p > 0), else fill 0
            nc.gpsimd.affine_select(
                out=ustrict[:], in_=ustrict[:], compare_op=ALU.is_gt,
                fill=0.0, base=0, pattern=[[1, P]], channel_multiplier=-1)
            onescol = consts.tile([P, 1], F32)
            nc.vector.memset(onescol[:], 1.0)
            onesrow = consts.tile([1, P], F32)
            nc.vector.memset(onesrow[:], 1.0)

            # ---- extract own-expert riders; weight = sigmoid(delta) ----
            bc3 = (P, TCH, E)
            esel3 = esel_t[:].unsqueeze(1).to_broadcast(bc3)
            selx = wb.tile([P, TCH, E], F32, tag="selx")
            nc.vector.tensor_tensor(selx[:], rid[:, :, :E], esel3, ALU.mult)
            sel_all = pb.tile([P, TCH], F32)
            nc.vector.tensor_reduce(sel_all[:], selx[:], AXL.X, ALU.add)
            dlx = wb.tile([P, TCH, E], F32, tag="dlx")
            nc.vector.tensor_tensor(dlx[:], rid[:, :, E:], esel3, ALU.mult)
            dla = wbs.tile([P, TCH], F32, tag="dla")
            nc.vector.tensor_reduce(dla[:], dlx[:], AXL.X, ALU.add)
            # prime the sigmoid act table early (scheduler runs this while
            # Act idles during the AllGather); consumed at zero weight below
            dsg = wbs.tile([P, 1], F32, tag="dsg")
            nc.scalar.activation(dsg[:], tokid_t[:, 0:1], AF.Sigmoid)
            z2 = wbs.tile([P, 1], F32, tag="z2")
            nc.vector.tensor_scalar(z2[:], dsg[:], 0.0, None, ALU.mult)
            wgt_all = pb.tile([P, TCH], F32)
            nc.scalar.activation(wgt_all[:], dla[:], AF.Sigmoid)
            nc.vector.tensor_tensor(wgt_all[:], wgt_all[:], sel_all[:],
                                    ALU.mult)
            if "dbg_sel" in tn:
                nc.sync.dma_start(tn["dbg_sel"][:], sel_all[:])

            # ---- global exclusive rank: within-chunk cumsum + chunk offs ---
            rank_ps = psb.tile([P, TCH], F32, tag="rank")
            _mm(nc, rank_ps[:], ustrict[:], sel_all[:], True, False)
            cnt_ps = psb.tile([TCH, 1], F32, tag="cnt")
            _mm(nc, cnt_ps[:], sel_all[:], onescol[:], True, True)
            cnt_sb = wbs.tile([TCH, 1], F32, tag="cnt_sb")
            nc.scalar.copy(cnt_sb[:], cnt_ps[:])
            offsrow_ps = psb.tile([1, TCH], F32, tag="offsrow")
            _mm(nc, offsrow_ps[:], cnt_sb[:], ustrict[:TCH, :TCH], True, True)
            offsrow_sb = wbs.tile([1, TCH], F32, tag="offsrow_sb")
            nc.scalar.copy(offsrow_sb[:], offsrow_ps[:])
            # broadcast chunk offsets across partitions via K=1 matmul,
            # accumulated straight into the rank psum
            _mm(nc, rank_ps[:], onesrow[:], offsrow_sb[:], False, True)

            # slot = rank where selected else huge (matches no iota column)
            slot_f = wb.tile([P, TCH], F32, tag="slotf")
            nc.vector.tensor_tensor(slot_f[:], rank_ps[:], sel_all[:],
                                    ALU.mult)
            big_f = wb.tile([P, TCH], F32, tag="bigf")
            nc.vector.tensor_scalar(big_f[:], sel_all[:], -1e6, 1e6,
                                    ALU.mult, ALU.add)
            nc.vector.tensor_tensor(slot_f[:], slot_f[:], big_f[:], ALU.add)

            # matmul-based compaction: one-hot(slot == iota) per chunk times
            # (w, tokid, occupied) accumulates the compact dispatch table
            # directly in slot-major layout -- no DRAM scatter round trip
            wt3 = pb.tile([P, TCH, 3], F32)
            nc.vector.tensor_copy(wt3[:, :, 0], wgt_all[:])
            nc.vector.tensor_copy(wt3[:, :, 1], tokid_t[:])
            nc.vector.tensor_scalar(wt3[:, :, 2], tokid_t[:], z2[:], 1.0,
                                    ALU.mult, ALU.add)
            psc = [psb.tile([P, 3], F32, tag=f"cmp{cb}", name=f"cmp{cb}")
                   for cb in range(CB)]
            for mtc in range(TCH):
                oh = wb.tile([P, C_CAP], F32, tag="oh")
                nc.vector.tensor_scalar(oh[:], iota_t[:],
                                        slot_f[:, mtc:mtc + 1], None,
                                        ALU.is_equal)
                for cb, (st, sz) in enumerate(CBS):
                    _mm(nc, psc[cb][:sz], oh[:, st:st + sz],
                        wt3[:, mtc], mtc == 0, mtc == TCH - 1)
            for cb, (st, sz) in enumerate(CBS):
                nc.vector.tensor_copy(wg_t[:sz, cb:cb + 1],
                                      psc[cb][:sz, 0:1])
                tfb = wbs.tile([P, 1], F32, tag="tfb")
                nc.vector.tensor_scalar(tfb[:sz], psc[cb][:sz, 2:3],
                                        -float(T), float(T), ALU.mult,
                                        ALU.add)
                idf = wbs.tile([P, 1], F32, tag="idf")
                nc.vector.tensor_tensor(idf[:sz], psc[cb][:sz, 1:2],
                                        tfb[:sz], ALU.add)
                nc.vector.tensor_copy(id_i[:sz, cb:cb + 1], idf[:sz])
            # gather normalized token rows (full-width rows: indirect DMA
            # needs AP width == tensor row stride); transpose to d-major
            h2r = pb.tile([P, CB, GWB], BF16)
            nc.vector.memset(h2r[:], 0.0)
            for cb, (st, sz) in enumerate(CBS):
                # trash ids (== T) are dropped by the bounds check and leave
                # the pre-zeroed row
                nc.gpsimd.indirect_dma_start(
                    out=h2r[:sz, cb], out_offset=None,
                    in_=xatt_d[:],
                    in_offset=bass.IndirectOffsetOnAxis(
                        ap=id_i[:sz, cb:cb + 1], axis=0),
                    bounds_check=T - 1, oob_is_err=False)
                for dc in range(DCH):
                    ptp = psbt.tile([P, P], BF16, tag="tp2")
                    nc.tensor.transpose(ptp[:, :sz],
                                        h2r[:sz, cb, dc * P:(dc + 1) * P],
                                        identb[:sz, :sz])
                    nc.scalar.copy(h2gT[:, dc, st:st + sz], ptp[:, :sz])
            if "dbg_h2" in tn:
                nc.sync.dma_start(
                    tn["dbg_h2"][:].rearrange("p (a b) -> p a b", b=D),
                    h2r[:, :, :D])

        # =================== phase C: expert FFN ===================
        with (
            tc.tile_pool(name="pc", bufs=1) as pc,
            tc.tile_pool(name="wc", bufs=3) as wc,
            tc.tile_pool(name="psf1", bufs=2, space="PSUM") as psf1,
            tc.tile_pool(name="psf2", bufs=2, space="PSUM") as psf2,
        ):
            b1T_t = consts.tile([P, FFCH], F32)
            nc.sync.dma_start(b1T_t[:], tn["b1T"][:])
            hT = pc.tile([P, FFCH, C_CAP], BF16)
            for mf in range(FFCH):
                w1_t = wc.tile([P, DCH, P], BF16, tag="w1s")
                nc.sync.dma_start(
                    w1_t[:], tn["w1"][mf].rearrange("p (o n) -> p o n", n=P))
                pt = psf1.tile([P, C_CAP], F32, tag="ffn1")
                for kd in range(DCH):
                    _mm(nc, pt[:], w1_t[:, kd], h2gT[:, kd], kd == 0,
                        kd == DCH - 1)
                nc.scalar.activation(hT[:, mf], pt[:], AF.Gelu_apprx_tanh,
                                     bias=b1T_t[:, mf:mf + 1])
                if mf == 0 and "dbg_hT" in tn:
                    nc.sync.dma_start(tn["dbg_hT"][:], hT[:, 0])

            # second matmul: per 128-token block, psum pair accumulates over
            # ff chunks from SBUF-resident w2; block outputs scatter while
            # the next block computes
            b2_t = consts.tile([P, D], F32)
            nc.sync.dma_start(b2_t[:], tn["b2"][:].to_broadcast((P, D)))
            for cb, (st, sz) in enumerate(CBS):
                psA = psf2.tile([P, 512], F32, tag="f2a")
                psB = psf2.tile([P, 512], F32, tag="f2b")
                for kf in range(FFCH):
                    _mm(nc, psA[:sz], hT[:, kf, st:st + sz],
                        w2sb[:, kf, :512], kf == 0, kf == FFCH - 1)
                for kf in range(FFCH):
                    _mm(nc, psB[:sz], hT[:, kf, st:st + sz],
                        w2sb[:, kf, 512:], kf == 0, kf == FFCH - 1)
                oew = wc.tile([P, D], BF16, tag="oew")
                nc.vector.tensor_tensor(oew[:sz, :512], psA[:sz],
                                        b2_t[:sz, :512], ALU.add)
                nc.vector.tensor_tensor(oew[:sz, 512:], psB[:sz],
                                        b2_t[:sz, 512:], ALU.add)
                nc.vector.tensor_scalar_mul(oew[:sz], oew[:sz],
                                            wg_t[:sz, cb:cb + 1])
                nc.gpsimd.indirect_dma_start(
                    out=partial_d[:],
                    out_offset=bass.IndirectOffsetOnAxis(
                        ap=id_i[:sz, cb:cb + 1], axis=0),
                    in_=oew[:sz], in_offset=None)

            # 8-core ReduceScatter of expert contributions + residual
            nc.gpsimd.collective_compute(
                "ReduceScatter", ALU.add,
                replica_groups=[[0, 1, 2, 3, 4, 5, 6, 7]],
                ins=[partial_d[:T, :].opt()], outs=[moe_sh[:].opt()])
            moe_t = wc.tile([P, D], BF16, tag="moet")
            nc.sync.dma_start(moe_t[:], moe_sh[:])
            out_t = wc.tile([P, D], F32, tag="outt")
            nc.vector.tensor_tensor(out_t[:], moe_t[:], xs_t[:], ALU.add)
            nc.sync.dma_start(tn["out_sh"][:], out_t[:])


_CACHED = {}


def _get_nc():
    if "nc" not in _CACHED:
        nc = bacc.Bacc("TRN2", target_bir_lowering=False, debug=False,
                       num_devices=NCORES)
        build(nc)
        nc.compile()
        _CACHED["nc"] = nc
    return _CACHED["nc"]


def make_in_maps(inputs):
    bf16 = ml_dtypes.bfloat16
    x = np.asarray(inputs["x"], np.float32)
    rope_cos = np.asarray(inputs["rope_cos"], np.float32)
    rope_sin = np.asarray(inputs["rope_sin"], np.float32)
    wq = np.asarray(inputs["wq"], np.float32)
    bq = np.asarray(inputs["bq"], np.float32)
    wk = np.asarray(inputs["wk"], np.float32)
    bk = np.asarray(inputs["bk"], np.float32)
    wv = np.asarray(inputs["wv"], np.float32)
    bv = np.asarray(inputs["bv"], np.float32)
    wo = np.asarray(inputs["wo"], np.float32)
    bo = np.asarray(inputs["bo"], np.float32)
    n1w = np.asarray(inputs["norm1_w"], np.float32)
    n2w = np.asarray(inputs["norm2_w"], np.float32)
    rw = np.asarray(inputs["router_w"], np.float32)
    rb = np.asarray(inputs["router_b"], np.float32)
    w1 = np.asarray(inputs["w1"], np.float32)
    b1 = np.asarray(inputs["b1"], np.float32)
    w2 = np.asarray(inputs["w2"], np.float32)
    b2 = np.asarray(inputs["b2"], np.float32)

    xf = x.reshape(T, D)
    xpb_full = (xf + bo[None, :]).astype(np.float32)
    mtri = np.where(np.arange(P)[:, None] >= np.arange(P)[None, :], 0.0,
                    -1e5).astype(np.float32)
    tokid = (np.arange(P)[:, None] + P * np.arange(TCH)[None, :]).astype(
        np.float32)
    iotac = np.arange(C_CAP, dtype=np.float32)[None, :]
    rw_scaled = (rw * n2w[:, None]).astype(np.float32)
    iscl = 1.0 / np.sqrt(HD)  # score scale folded into wq/bq
    wqn = (wq * n1w[:, None] * iscl).astype(np.float32)
    wkn = (wk * n1w[:, None]).astype(np.float32)
    wvn = (wv * n1w[:, None]).astype(np.float32)
    # packed router weights: rw_packed[p, kd*E+e] = rw_scaled[kd*128+p, e]
    rw_packed = np.ascontiguousarray(
        rw_scaled.reshape(DCH, P, E).transpose(1, 0, 2).reshape(P, DCH * E))
    cos2T = np.ascontiguousarray(np.tile(rope_cos.T, (2, 1)).astype(bf16))
    sin2T = np.ascontiguousarray(np.tile(rope_sin.T, (2, 1)).astype(bf16))
    # rot_half as matmul: out[m] = sum_k rotm[k, m] * in[k] per 64-block
    r64 = np.zeros((HD, HD), np.float32)
    for m in range(HD // 2):
        r64[m + HD // 2, m] = -1.0
    for m in range(HD // 2, HD):
        r64[m - HD // 2, m] = 1.0
    rotm = np.zeros((P, P), bf16)
    rotm[:HD, :HD] = r64
    rotm[HD:, HD:] = r64
    # w1 pre-permuted (n2w folded in):
    # w1h[c][mf, p, kd*128+f] = n2w[kd*128+p] * w1[c][kd*128+p, mf*128+f]
    w1n = w1 * n2w[None, :, None]
    w1h = [np.ascontiguousarray(
        w1n[c].reshape(DCH, P, FFCH, P).transpose(2, 1, 0, 3).reshape(
            FFCH, P, D).astype(bf16)) for c in range(NCORES)]

    in_maps = []
    for c in range(NCORES):
        b, g = c // 4, c % 4
        esel = np.zeros((1, E), np.float32)
        esel[0, c] = 1.0
        in_maps.append({
            "xb": np.ascontiguousarray(x[b].astype(bf16)),
            "xpb": np.ascontiguousarray(xpb_full[c * P:(c + 1) * P]),
            "cosT": cos2T,
            "sinT": sin2T,
            "rotm": rotm,
            "wq": np.ascontiguousarray(
                wqn[:, g * 4 * HD:(g + 1) * 4 * HD].astype(bf16)),
            "wk": np.ascontiguousarray(np.tile(
                wkn[:, g * HD:(g + 1) * HD], (1, 2)).astype(bf16)),
            "wv": np.ascontiguousarray(
                wvn[:, g * HD:(g + 1) * HD].astype(bf16)),
            "bq": np.ascontiguousarray(
                (bq[g * 4 * HD:(g + 1) * 4 * HD] * iscl).reshape(2, P).T),
            "bk": np.ascontiguousarray(
                np.tile(bk[g * HD:(g + 1) * HD], 2)[:, None]),
            "bv": np.ascontiguousarray(bv[None, g * HD:(g + 1) * HD]),
            "wo": np.ascontiguousarray(
                wo[g * 4 * HD:(g + 1) * 4 * HD, :].astype(bf16)),
            "rw": rw_packed,
            "rb": np.ascontiguousarray(rb[None, :]),
            "xpbrw": np.ascontiguousarray(
                xpb_full[c * P:(c + 1) * P] @ rw_scaled),
            "mtri": mtri,
            "w1": w1h[c],
            "b1T": np.ascontiguousarray(b1[c].reshape(FFCH, P).T),
            "w2": np.ascontiguousarray(w2[c].astype(bf16)),
            "b2": np.ascontiguousarray(b2[c][None, :]),
            "tokid": tokid,
            "iotac": iotac,
            "esel": esel,
        })
    return in_maps


def kernel(**inputs) -> np.ndarray:
    in_maps = make_in_maps(inputs)
    nc = _get_nc()
    res = bass_utils.run_bass_kernel_spmd(nc, in_maps,
                                          core_ids=list(range(NCORES)))
    out = np.concatenate([res.results[c]["out_sh"] for c in range(NCORES)], 0)
    return out.reshape(B, S, D)


# revision 107
# speedup vs baseline: 1.0149x; 1.0149x over previous
"""Trainium2 Bass kernel for a decoder layer (GQA attention + top-2 MoE FFN).

Sharding over 8 NeuronCores (one SPMD NEFF, per-core input data differs):
  - Attention: core c handles (batch b=c//4, kv-group g=c%4): 4 query heads,
    1 kv head, and the matching out-proj row-slice. Partials are combined
    with a 4-core ReduceScatter (token-sharded); each core adds bias +
    residual for its 128-token shard and computes that shard's router
    logits. The post-attention state is downcast to bf16 (logits ride
    bit-exact as bf16 pairs in columns 1024..1039) and an 8-core
    AllGather (Shared output) gives every core the full [T, 1040] state.
  - MoE: expert-parallel, core c owns expert e=c. Top-2 routing is
    recomputed (replicated, vectorized over all 8 token chunks at once)
    from the shared logits; ranks come from one triangular-matmul cumsum
    plus a chunk-offset fixup. Each core scatters only 8-byte
    (weight, tokid) records into a compact table, indirect-GATHERS its
    expert's <=C_CAP token rows (bf16) straight from the AllGather
    buffer, RMS-normalizes them, runs the dense FFN (bf16 weights, w2
    preloaded in SBUF so block outputs scatter while the next block
    computes), scatters weighted bf16 outputs back to token rows of a
    zeroed [T, D] partial buffer, and an 8-core ReduceScatter sums the
    expert contributions. Each core emits its 128-token output shard;
    the host concatenates shards into the full [B, S, D] output.
"""
import os

import ml_dtypes
import numpy as np

import concourse.bass as bass
import concourse.mybir as mybir
import concourse.tile as tile
from concourse import bacc
from concourse import bass_utils
from concourse.masks import make_identity

# model dims (hardcoded per problem spec)
B, S, D = 2, 512, 1024
H, KV, HD = 16, 4, 64
E, FF, TOPK = 8, 4096, 2
EPS = 1e-6
T = B * S          # 1024 tokens
P = 128
NCORES = 8
C_CAP = 320        # per-expert token capacity (mean 256, +4.6 sigma)
CB = 3             # capacity blocks (ragged: 128 + 128 + 64)
CBS = [(0, P), (P, P), (2 * P, C_CAP - 2 * P)]  # (start, size) per block
DCH = D // P       # 8
FFCH = FF // P     # 32
TCH = T // P       # 8
SB = S // P        # 4
GWB = D + 16       # bf16 AG row: 1024 data + 8 f32 logits as 16 bf16

F32 = mybir.dt.float32
BF16 = mybir.dt.bfloat16
I32 = mybir.dt.int32
AF = mybir.ActivationFunctionType
ALU = mybir.AluOpType
AXL = mybir.AxisListType


def _mm(nc, out, lhsT, rhs, start, stop, dt=None):
    if dt is not None and dt != F32:
        if lhsT.dtype != dt:
            lhsT = lhsT.bitcast(dt)
        if rhs.dtype != dt:
            rhs = rhs.bitcast(dt)
    nc.tensor.matmul(out, lhsT=lhsT, rhs=rhs, start=start, stop=stop)


def build(nc: bass.Bass):
    def dram(n, s, d=F32):
        return nc.dram_tensor(n, s, d, kind="ExternalInput")

    tn = {}
    tn["xb"] = dram("xb", [S, D], BF16)      # x[b] for this core's batch
    tn["xpb"] = dram("xpb", [P, D])          # (x + bo) rows [c*128:(c+1)*128]
    tn["cosT"] = dram("cosT", [P, S], BF16)  # rope cos^T duplicated rows
    tn["sinT"] = dram("sinT", [P, S], BF16)
    tn["rotm"] = dram("rotm", [P, P], BF16)  # rot_half as matmul lhsT
    tn["wq"] = dram("wq", [D, 4 * HD], BF16)  # this core's 4 query heads
    tn["wk"] = dram("wk", [D, 2 * HD], BF16)  # kv head dup to both halves
    tn["wv"] = dram("wv", [D, HD], BF16)
    tn["bq"] = dram("bq", [P, 2])
    tn["bk"] = dram("bk", [2 * HD, 1])
    tn["bv"] = dram("bv", [1, HD])
    tn["wo"] = dram("wo", [4 * HD, D], BF16)  # rows g*256..(g+1)*256 of wo
    tn["rw"] = dram("rw", [P, DCH * E])      # (router_w*norm2_w) packed
    tn["rb"] = dram("rb", [1, E])
    tn["xpbrw"] = dram("xpbrw", [P, E])      # (x+bo) shard @ rw, host-side
    tn["mtri"] = dram("mtri", [P, P])        # additive causal mask (0/-1e5)
    tn["w1"] = dram("w1", [FFCH, P, D], BF16)  # w1h[mf,p,kd*128+f]
    tn["b1T"] = dram("b1T", [P, FFCH])
    tn["w2"] = dram("w2", [FF, D], BF16)
    tn["b2"] = dram("b2", [1, D])
    tn["tokid"] = dram("tokid", [P, TCH])    # tc*128+p as f32
    tn["iotac"] = dram("iotac", [1, C_CAP])  # 0..C_CAP-1 as f32
    tn["esel"] = dram("esel", [1, E])        # one-hot row for expert e
    tn["out_sh"] = nc.dram_tensor("out_sh", [P, D], F32, kind="ExternalOutput")
    if os.environ.get("KDBG") == "1":
        out = lambda n, s, d=F32: nc.dram_tensor(n, s, d, kind="ExternalOutput")
        tn["dbg_xs"] = out("dbg_xs", [P, D])
        tn["dbg_sel"] = out("dbg_sel", [P, TCH])
        tn["dbg_h2"] = out("dbg_h2", [P, CB * D], BF16)
        tn["dbg_hT"] = out("dbg_hT", [P, C_CAP], BF16)

    with tile.TileContext(nc) as tc:
        _build_tc(nc, tc, tn)
    return nc


def _build_tc(nc, tc, tn):
    with (
        tc.tile_pool(name="consts", bufs=1) as consts,
        tc.tile_pool(name="persist", bufs=1) as persist,
        tc.tile_pool(name="dram", bufs=1, space="DRAM") as dpool,
    ):
        ident = consts.tile([P, P], F32)
        make_identity(nc, ident[:])
        identb = consts.tile([P, P], BF16)
        make_identity(nc, identb[:])

        # w2 preload buffer (DMA issued after attention weight loads)
        w2sb = persist.tile([P, FFCH, D], BF16)

        # ---- DRAM buffers (zero/init DMAs issued later, in phase B) ----
        zerob = consts.tile([P, D], BF16)
        nc.vector.memset(zerob[:], 0.0)
        partial_d = dpool.tile([T + P, D], BF16)    # rows T.. = trash
        po_d = dpool.tile([S, D], BF16)
        rs_att = dpool.tile([P, D], BF16)
        # AG rows: 1024 normalized-h2 + 8 top-2 sel + 8 weight-delta riders
        xs_d = dpool.tile([P, GWB], BF16)
        xatt_d = dpool.tile([T, GWB], BF16, addr_space="Shared")
        moe_sh = dpool.tile([P, D], BF16)

        # long-lived SBUF
        xs_t = persist.tile([P, D], F32)            # shard residual state
        h2gT = persist.tile([P, DCH, C_CAP], BF16)  # compacted tokens (d-maj)
        wg_t = persist.tile([P, CB], F32)
        id_i = persist.tile([P, CB], I32)

        # =================== phase A: attention ===================
        with (
            tc.tile_pool(name="pa", bufs=1) as pa,
            tc.tile_pool(name="wa", bufs=3) as wa,
            tc.tile_pool(name="was", bufs=3) as was,
            tc.tile_pool(name="ps512", bufs=3, space="PSUM") as ps512,
            tc.tile_pool(name="pstp", bufs=2, space="PSUM") as pstp,
            tc.tile_pool(name="pssm", bufs=3, space="PSUM") as pssm,
        ):
            def transpose_to(dst_ap, src_ap, idn, cp=None):
                dt = idn.dtype
                pt = pstp.tile([P, P], dt,
                               tag="tpf" if dt == F32 else "tpb")
                nc.tensor.transpose(pt[:], src_ap, idn)
                (cp or nc.scalar.copy)(dst_ap, pt[:])

            x_t = pa.tile([P, SB, D], BF16)
            for tb in range(SB):
                nc.sync.dma_start(x_t[:, tb], tn["xb"][tb * P:(tb + 1) * P])

            # rms norm 1 -> h1 (token layout)
            h1_t = pa.tile([P, SB, D], BF16)
            for tb in range(SB):
                sq = wa.tile([P, D], F32, tag="sq")
                ssq = was.tile([P, 1], F32, tag="ssq")
                nc.scalar.activation(sq[:], x_t[:, tb], AF.Square,
                                     accum_out=ssq[:])
                ms = was.tile([P, 1], F32, tag="ms")
                nc.vector.tensor_scalar(ms[:], ssq[:], 1.0 / D, EPS,
                                        ALU.mult, ALU.add)
                rinv = was.tile([P, 1], F32, tag="rinv")
                nc.vector.reciprocal(rinv[:], ms[:])
                rsq = was.tile([P, 1], F32, tag="rsq")
                nc.scalar.sqrt(rsq[:], rinv[:])
                nc.vector.tensor_scalar_mul(h1_t[:, tb], x_t[:, tb], rsq[:])

            # transpose h1 -> h1T [p=d, dc, tok]
            h1T = pa.tile([P, DCH, S], BF16)
            for tb in range(SB):
                for dc in range(DCH):
                    transpose_to(h1T[:, dc, tb * P:(tb + 1) * P],
                                 h1_t[:, tb, dc * P:(dc + 1) * P], identb[:],
                                 cp=nc.vector.tensor_copy)

            # q projection -> qT [p, m, tok]
            wq_t = pa.tile([P, DCH, 4 * HD], BF16)
            nc.sync.dma_start(wq_t[:],
                              tn["wq"][:].rearrange("(o p) n -> p o n", p=P))
            bq_t = pa.tile([P, 2], F32)
            nc.sync.dma_start(bq_t[:], tn["bq"][:])
            qT = pa.tile([P, 2, S], BF16)
            for m in range(2):
                pt = ps512.tile([P, 512], F32, tag="mm512")
                for kd in range(DCH):
                    _mm(nc, pt[:], wq_t[:, kd, m * P:(m + 1) * P], h1T[:, kd],
                        kd == 0, kd == DCH - 1)
                nc.scalar.activation(qT[:, m], pt[:], AF.Identity,
                                     bias=bq_t[:, m:m + 1])

            # k projection (kv head duplicated to both halves) -> kT [128, S]
            wk_t = pa.tile([P, DCH, 2 * HD], BF16)
            nc.sync.dma_start(wk_t[:],
                              tn["wk"][:].rearrange("(o p) n -> p o n", p=P))
            bk_t = pa.tile([2 * HD, 1], F32)
            nc.sync.dma_start(bk_t[:], tn["bk"][:])
            kT = pa.tile([P, S], BF16)
            ptk = ps512.tile([P, 512], F32, tag="mm512")
            for kd in range(DCH):
                _mm(nc, ptk[:], wk_t[:, kd], h1T[:, kd], kd == 0,
                    kd == DCH - 1)
            nc.scalar.activation(kT[:], ptk[:], AF.Identity,
                                 bias=bk_t[:, 0:1])

            # v projection -> v_t [p=tok, tb, 64] (token layout)
            wv_t = pa.tile([P, DCH, HD], BF16)
            nc.sync.dma_start(wv_t[:],
                              tn["wv"][:].rearrange("(o p) n -> p o n", p=P))
            bv_t = pa.tile([P, HD], F32)
            nc.sync.dma_start(bv_t[:], tn["bv"][:].to_broadcast((P, HD)))
            v_t = pa.tile([P, SB, HD], BF16)
            for tb in range(SB):
                pt = pssm.tile([P, HD], F32, tag="sm")
                for kd in range(DCH):
                    _mm(nc, pt[:], h1T[:, kd, tb * P:(tb + 1) * P],
                        wv_t[:, kd], kd == 0, kd == DCH - 1)
                nc.vector.tensor_tensor(v_t[:, tb], pt[:], bv_t[:],
                                        ALU.add)

            # rope: rot_half via rotation-matrix matmul (no partition shifts)
            cos_t = consts.tile([P, S], BF16)
            sin_t = consts.tile([P, S], BF16)
            nc.sync.dma_start(cos_t[:], tn["cosT"][:])
            nc.sync.dma_start(sin_t[:], tn["sinT"][:])
            rotm_t = consts.tile([P, P], BF16)
            nc.sync.dma_start(rotm_t[:], tn["rotm"][:])

            def rope(dst):  # dst: [128, S] AP (two 64-d groups), in place
                ptr_ = ps512.tile([P, S], F32, tag="mm512")
                _mm(nc, ptr_[:], rotm_t[:], dst, True, True)
                t1 = wa.tile([P, S], BF16, tag="ropet1")
                nc.vector.tensor_tensor(t1[:], dst, cos_t[:], ALU.mult)
                t2 = wa.tile([P, S], BF16, tag="ropet2")
                nc.vector.tensor_tensor(t2[:], ptr_[:], sin_t[:], ALU.mult)
                nc.vector.tensor_tensor(dst, t1[:], t2[:], ALU.add)

            for m in range(2):
                rope(qT[:, m])
            rope(kT[:])

            # scores -> softmax -> AV per head / query block
            mtri_t = consts.tile([P, P], F32)
            nc.sync.dma_start(mtri_t[:], tn["mtri"][:])
            o_t = pa.tile([P, SB, 4 * HD], BF16)
            oT = pa.tile([P, 2, S], BF16)
            wo_t = pa.tile([P, 2, D], BF16)
            nc.sync.dma_start(wo_t[:],
                              tn["wo"][:].rearrange("(o p) n -> p o n", p=P))
            # w2 preload: issued after every attention-critical DMA so it
            # fills the DMA engines only during compute / collectives
            nc.sync.dma_start(w2sb[:],
                              tn["w2"][:].rearrange("(o p) d -> p o d", p=P))
            for i in range(SB):
                for h in range(4):
                    nk = (i + 1) * P
                    hb = (h % 2) * HD
                    q_ap = qT[hb:hb + HD, h // 2, i * P:(i + 1) * P]
                    ps_s = ps512.tile([P, 512], F32, tag="mm512")
                    _mm(nc, ps_s[:, :nk], q_ap, kT[hb:hb + HD, :nk], True,
                        True)
                    nc.vector.tensor_tensor(ps_s[:, i * P:nk],
                                            ps_s[:, i * P:nk],
                                            mtri_t[:], ALU.add)
                    nm = was.tile([P, 1], F32, tag="negmax")
                    nc.vector.tensor_reduce(nm[:], ps_s[:, :nk], AXL.X,
                                            ALU.max, negate=True)
                    pr = wa.tile([P, 512], BF16, tag="probs")
                    ssum = was.tile([P, 1], F32, tag="ssum")
                    nc.scalar.activation(pr[:, :nk], ps_s[:, :nk], AF.Exp,
                                         bias=nm[:], accum_out=ssum[:])
                    rs = was.tile([P, 1], F32, tag="rsum")
                    nc.vector.reciprocal(rs[:], ssum[:])
                    ps_o = pssm.tile([P, HD], F32, tag="sm")
                    for j in range(i + 1):
                        pT = wa.tile([P, P], BF16, tag="pT")
                        transpose_to(pT[:], pr[:, j * P:(j + 1) * P],
                                     identb[:], cp=nc.vector.tensor_copy)
                        _mm(nc, ps_o[:], pT[:], v_t[:, j], j == 0, j == i)
                    nc.vector.tensor_scalar_mul(
                        o_t[:, i, h * HD:(h + 1) * HD], ps_o[:], rs[:])

                # token block i complete: transpose + out-projection + DMA
                tb = i
                for m in range(2):
                    transpose_to(oT[:, m, tb * P:(tb + 1) * P],
                                 o_t[:, tb, m * P:(m + 1) * P], identb[:])
                for nh in range(2):
                    pt = ps512.tile([P, 512], F32, tag="mm512")
                    for ko in range(2):
                        _mm(nc, pt[:], oT[:, ko, tb * P:(tb + 1) * P],
                            wo_t[:, ko, nh * 512:(nh + 1) * 512],
                            ko == 0, ko == 1)
                    po_sb = wa.tile([P, 512], BF16, tag="posb")
                    nc.vector.tensor_copy(po_sb[:], pt[:])
                    nc.sync.dma_start(
                        po_d[tb * P:(tb + 1) * P, nh * 512:(nh + 1) * 512],
                        po_sb[:])

            # prime the Act sqrt function table right after the last softmax
            # exp (real sqrt below then skips the ~1.3us table swap); the
            # result feeds the logit chain with zero weight so it is neither
            # dead code nor hoistable before the exps
            dumm = was.tile([P, 1], F32, tag="dumm")
            nc.scalar.sqrt(dumm[:], ssum[:])
            zflag = was.tile([P, 1], F32, tag="zflag")
            nc.vector.tensor_scalar(zflag[:], dumm[:], 0.0, None, ALU.mult)

            # 4-core ReduceScatter within batch group -> 128-token shard
            nc.gpsimd.collective_compute(
                "ReduceScatter", ALU.add,
                replica_groups=[[0, 1, 2, 3], [4, 5, 6, 7]],
                ins=[po_d[:].opt()], outs=[rs_att[:].opt()])

            # shard: add residual + bo; compute shard router logits; pack.
            # logits = rsq * (rsb@rwn + xpb@rwn) + rb with xpb@rwn host-side
            rsb = wa.tile([P, D], BF16, tag="rsb")
            nc.sync.dma_start(rsb[:], rs_att[:])
            xpb_t = wa.tile([P, D], F32, tag="probs2")
            nc.sync.dma_start(xpb_t[:], tn["xpb"][:])
            nc.vector.tensor_tensor(xs_t[:], rsb[:], xpb_t[:], ALU.add)

            rsT = pa.tile([P, DCH, P], F32)   # bf16 values lifted exactly
            for dc in range(DCH):
                transpose_to(rsT[:, dc], rsb[:, dc * P:(dc + 1) * P],
                             identb[:])
            sq = wa.tile([P, D], F32, tag="sq")
            ssq = was.tile([P, 1], F32, tag="ssq")
            nc.scalar.activation(sq[:], xs_t[:], AF.Square,
                                 accum_out=ssq[:])
            ms = was.tile([P, 1], F32, tag="ms")
            nc.vector.tensor_scalar(ms[:], ssq[:], 1.0 / D, EPS, ALU.mult,
                                    ALU.add)
            rinv = was.tile([P, 1], F32, tag="rinv")
            nc.vector.reciprocal(rinv[:], ms[:])
            rsq = was.tile([P, 1], F32, tag="rsq")
            nc.scalar.sqrt(rsq[:], rinv[:])
            rw_t = consts.tile([P, DCH, E], F32)
            nc.sync.dma_start(rw_t[:], tn["rw"][:].rearrange(
                "p (o n) -> p o n", n=E))
            rb_t = consts.tile([P, E], F32)
            nc.sync.dma_start(rb_t[:], tn["rb"][:].to_broadcast((P, E)))
            xpbrw_t = consts.tile([P, E], F32)
            nc.sync.dma_start(xpbrw_t[:], tn["xpbrw"][:])
            ptl = pssm.tile([P, HD], F32, tag="sm")
            for dc in range(DCH):
                _mm(nc, ptl[:, :E], rsT[:, dc], rw_t[:, dc], dc == 0,
                    dc == DCH - 1)
            lg = was.tile([P, E], F32, tag="lg")
            nc.vector.tensor_tensor(lg[:], ptl[:, :E], xpbrw_t[:], ALU.add)
            nc.vector.tensor_scalar(lg[:], lg[:], rsq[:], zflag[:],
                                    ALU.mult, ALU.add)
            nc.vector.tensor_tensor(lg[:], lg[:], rb_t[:], ALU.add)
            # shard-local top-2 (monotone in logits, no exp needed) and the
            # weight delta: w_e = sigmoid(2*lg_e - v1 - v2), applied post-AG
            v1n = was.tile([P, 1], F32, tag="v1n")
            nc.vector.tensor_reduce(v1n[:], lg[:], AXL.X, ALU.max)
            s1 = was.tile([P, E], F32, tag="s1a")
            nc.vector.tensor_scalar(s1[:], lg[:], v1n[:], None, ALU.is_equal)
            e2m = was.tile([P, E], F32, tag="e2m")
            nc.vector.tensor_scalar(e2m[:], s1[:], -1e9, None, ALU.mult)
            nc.vector.tensor_tensor(e2m[:], lg[:], e2m[:], ALU.add)
            v2n = was.tile([P, 1], F32, tag="v2n")
            nc.vector.tensor_reduce(v2n[:], e2m[:], AXL.X, ALU.max)
            s2 = was.tile([P, E], F32, tag="s2a")
            nc.vector.tensor_scalar(s2[:], e2m[:], v2n[:], None,
                                    ALU.is_equal)
            selr = was.tile([P, E], F32, tag="selr")
            nc.vector.tensor_tensor(selr[:], s1[:], s2[:], ALU.add)
            vs = was.tile([P, 1], F32, tag="vs")
            nc.vector.tensor_tensor(vs[:], v1n[:], v2n[:], ALU.add)
            dl = was.tile([P, E], F32, tag="dl")
            nc.vector.tensor_scalar(dl[:], lg[:], 2.0, None, ALU.mult)
            nc.vector.tensor_scalar(dl[:], dl[:], vs[:], None, ALU.subtract)
            # normalized h2 shard (norm2_w folded into w1 host-side)
            xsb = pa.tile([P, GWB], BF16)
            nc.vector.tensor_scalar_mul(xsb[:, :D], xs_t[:], rsq[:])
            nc.vector.tensor_copy(xsb[:, D:D + E], selr[:])
            nc.vector.tensor_copy(xsb[:, D + E:D + 2 * E], dl[:])
            nc.sync.dma_start(xs_d[:], xsb[:])
            if "dbg_xs" in tn:
                nc.sync.dma_start(tn["dbg_xs"][:], xs_t[:])

        # 8-core AllGather: normalized h2 + routing riders (bf16 rows)
        nc.gpsimd.collective_compute(
            "AllGather", ALU.bypass,
            replica_groups=[[0, 1, 2, 3, 4, 5, 6, 7]],
            ins=[xs_d[:].opt()], outs=[xatt_d[:].opt()])

        # =================== phase B: routing + dispatch ===================
        with (
            tc.tile_pool(name="pb", bufs=1) as pb,
            tc.tile_pool(name="wb", bufs=2) as wb,
            tc.tile_pool(name="wbs", bufs=3) as wbs,
            tc.tile_pool(name="psb", bufs=1, space="PSUM") as psb,
            tc.tile_pool(name="psbt", bufs=2, space="PSUM") as psbt,
        ):
            # routing riders for all tokens: [p, chunk, 2E] bf16
            rid = pb.tile([P, TCH, 2 * E], BF16)
            nc.sync.dma_start(
                rid[:],
                xatt_d[:, D:D + 2 * E].rearrange("(o p) d -> p o d", p=P))
            tokid_t = consts.tile([P, TCH], F32)
            nc.sync.dma_start(tokid_t[:], tn["tokid"][:])
            iota_t = consts.tile([P, C_CAP], F32)
            nc.sync.dma_start(iota_t[:], tn["iotac"][:].to_broadcast(
                (P, C_CAP)))
            # deferred DRAM zeroing: needed by the FFN2 scatters
            for i_ in range(TCH):
                nc.sync.dma_start(partial_d[i_ * P:(i_ + 1) * P, :],
                                  zerob[:])
            esel_t = consts.tile([P, E], F32)
            nc.sync.dma_start(esel_t[:], tn["esel"][:].to_broadcast((P, E)))
            ustrict = consts.tile([P, P], F32)
            nc.vector.memset(ustrict[:], 1.0)
            # keep 1.0 where p < f (iota = f - p > 0), else fill 0
            nc.gpsimd.affine_select(
                out=ustrict[:], in_=ustrict[:], compare_op=ALU.is_gt,
                fill=0.0, base=0, pattern=[[1, P]], channel_multiplier=-1)
            onescol = consts.tile([P, 1], F32)
            nc.vector.memset(onescol[:], 1.0)
            onesrow = consts.tile([1, P], F32)
            nc.vector.memset(onesrow[:], 1.0)

            # ---- extract own-expert riders; weight = sigmoid(delta) ----
            bc3 = (P, TCH, E)
            esel3 = esel_t[:].unsqueeze(1).to_broadcast(bc3)
            selx = wb.tile([P, TCH, E], F32, tag="selx")
            nc.vector.tensor_tensor(selx[:], rid[:, :, :E], esel3, ALU.mult)
            sel_all = pb.tile([P, TCH], F32)
            nc.vector.tensor_reduce(sel_all[:], selx[:], AXL.X, ALU.add)
            dlx = wb.tile([P, TCH, E], F32, tag="dlx")
            nc.vector.tensor_tensor(dlx[:], rid[:, :, E:], esel3, ALU.mult)
            dla = wbs.tile([P, TCH], F32, tag="dla")
            nc.vector.tensor_reduce(dla[:], dlx[:], AXL.X, ALU.add)
            # prime the sigmoid act table early (scheduler runs this while
            # Act idles during the AllGather); consumed at zero weight below
            dsg = wbs.tile([P, 1], F32, tag="dsg")
            nc.scalar.activation(dsg[:], tokid_t[:, 0:1], AF.Sigmoid)
            z2 = wbs.tile([P, 1], F32, tag="z2")
            nc.vector.tensor_scalar(z2[:], dsg[:], 0.0, None, ALU.mult)
            wgt_all = pb.tile([P, TCH], F32)
            nc.scalar.activation(wgt_all[:], dla[:], AF.Sigmoid)
            nc.vector.tensor_tensor(wgt_all[:], wgt_all[:], sel_all[:],
                                    ALU.mult)
            if "dbg_sel" in tn:
                nc.sync.dma_start(tn["dbg_sel"][:], sel_all[:])

            # ---- global exclusive rank: within-chunk cumsum + chunk offs ---
            rank_ps = psb.tile([P, TCH], F32, tag="rank")
            _mm(nc, rank_ps[:], ustrict[:], sel_all[:], True, False)
            cnt_ps = psb.tile([TCH, 1], F32, tag="cnt")
            _mm(nc, cnt_ps[:], sel_all[:], onescol[:], True, True)
            cnt_sb = wbs.tile([TCH, 1], F32, tag="cnt_sb")
            nc.scalar.copy(cnt_sb[:], cnt_ps[:])
            offsrow_ps = psb.tile([1, TCH], F32, tag="offsrow")
            _mm(nc, offsrow_ps[:], cnt_sb[:], ustrict[:TCH, :TCH], True, True)
            offsrow_sb = wbs.tile([1, TCH], F32, tag="offsrow_sb")
            nc.scalar.copy(offsrow_sb[:], offsrow_ps[:])
            # broadcast chunk offsets across partitions via K=1 matmul,
            # accumulated straight into the rank psum
            _mm(nc, rank_ps[:], onesrow[:], offsrow_sb[:], False, True)

            # slot = rank where selected else huge (matches no iota column)
            slot_f = wb.tile([P, TCH], F32, tag="slotf")
            nc.vector.tensor_tensor(slot_f[:], rank_ps[:], sel_all[:],
                                    ALU.mult)
            big_f = wb.tile([P, TCH], F32, tag="bigf")
            nc.vector.tensor_scalar(big_f[:], sel_all[:], -1e6, 1e6,
                                    ALU.mult, ALU.add)
            nc.vector.tensor_tensor(slot_f[:], slot_f[:], big_f[:], ALU.add)

            # matmul-based compaction: one-hot(slot == iota) per chunk times
            # (w, tokid, occupied) accumulates the compact dispatch table
            # directly in slot-major layout -- no DRAM scatter round trip
            wt3 = pb.tile([P, TCH, 3], F32)
            nc.vector.tensor_copy(wt3[:, :, 0], wgt_all[:])
            nc.vector.tensor_copy(wt3[:, :, 1], tokid_t[:])
            nc.vector.tensor_scalar(wt3[:, :, 2], tokid_t[:], z2[:], 1.0,
                                    ALU.mult, ALU.add)
            psc = [psb.tile([P, 3], F32, tag=f"cmp{cb}", name=f"cmp{cb}")
                   for cb in range(CB)]
            for mtc in range(TCH):
                oh = wb.tile([P, C_CAP], F32, tag="oh")
                nc.vector.tensor_scalar(oh[:], iota_t[:],
                                        slot_f[:, mtc:mtc + 1], None,
                                        ALU.is_equal)
                for cb, (st, sz) in enumerate(CBS):
                    _mm(nc, psc[cb][:sz], oh[:, st:st + sz],
                        wt3[:, mtc], mtc == 0, mtc == TCH - 1)
            for cb, (st, sz) in enumerate(CBS):
                nc.vector.tensor_copy(wg_t[:sz, cb:cb + 1],
                                      psc[cb][:sz, 0:1])
                tfb = wbs.tile([P, 1], F32, tag="tfb")
                nc.vector.tensor_scalar(tfb[:sz], psc[cb][:sz, 2:3],
                                        -float(T), float(T), ALU.mult,
                                        ALU.add)
                idf = wbs.tile([P, 1], F32, tag="idf")
                nc.vector.tensor_tensor(idf[:sz], psc[cb][:sz, 1:2],
                                        tfb[:sz], ALU.add)
                nc.vector.tensor_copy(id_i[:sz, cb:cb + 1], idf[:sz])
            # gather normalized token rows (full-width rows: indirect DMA
            # needs AP width == tensor row stride); transpose to d-major
            h2r = pb.tile([P, CB, GWB], BF16)
            nc.vector.memset(h2r[:], 0.0)
            for cb, (st, sz) in enumerate(CBS):
                # trash ids (== T) are dropped by the bounds check and leave
                # the pre-zeroed row
                nc.gpsimd.indirect_dma_start(
                    out=h2r[:sz, cb], out_offset=None,
                    in_=xatt_d[:],
                    in_offset=bass.IndirectOffsetOnAxis(
                        ap=id_i[:sz, cb:cb + 1], axis=0),
                    bounds_check=T - 1, oob_is_err=False)
                for dc in range(DCH):
                    ptp = psbt.tile([P, P], BF16, tag="tp2")
                    nc.tensor.transpose(ptp[:, :sz],
                                        h2r[:sz, cb, dc * P:(dc + 1) * P],
                                        identb[:sz, :sz])
                    nc.scalar.copy(h2gT[:, dc, st:st + sz], ptp[:, :sz])
            if "dbg_h2" in tn:
                nc.sync.dma_start(
                    tn["dbg_h2"][:].rearrange("p (a b) -> p a b", b=D),
                    h2r[:, :, :D])

        # =================== phase C: expert FFN ===================
        with (
            tc.tile_pool(name="pc", bufs=1) as pc,
            tc.tile_pool(name="wc", bufs=3) as wc,
            tc.tile_pool(name="psf1", bufs=2, space="PSUM") as psf1,
            tc.tile_pool(name="psf2", bufs=2, space="PSUM") as psf2,
        ):
            b1T_t = consts.tile([P, FFCH], F32)
            nc.sync.dma_start(b1T_t[:], tn["b1T"][:])
            hT = pc.tile([P, FFCH, C_CAP], BF16)
            for mf in range(FFCH):
                w1_t = wc.tile([P, DCH, P], BF16, tag="w1s")
                nc.sync.dma_start(
                    w1_t[:], tn["w1"][mf].rearrange("p (o n) -> p o n", n=P))
                pt = psf1.tile([P, C_CAP], F32, tag="ffn1")
                for kd in range(DCH):
                    _mm(nc, pt[:], w1_t[:, kd], h2gT[:, kd], kd == 0,
                        kd == DCH - 1)
                nc.scalar.activation(hT[:, mf], pt[:], AF.Gelu_apprx_tanh,
                                     bias=b1T_t[:, mf:mf + 1])
                if mf == 0 and "dbg_hT" in tn:
                    nc.sync.dma_start(tn["dbg_hT"][:], hT[:, 0])

            # second matmul: per 128-token block, psum pair accumulates over
            # ff chunks from SBUF-resident w2; block outputs scatter while
            # the next block computes
            b2_t = consts.tile([P, D], F32)
            nc.sync.dma_start(b2_t[:], tn["b2"][:].to_broadcast((P, D)))
            for cb, (st, sz) in enumerate(CBS):
                psA = psf2.tile([P, 512], F32, tag="f2a")
                psB = psf2.tile([P, 512], F32, tag="f2b")
                for kf in range(FFCH):
                    _mm(nc, psA[:sz], hT[:, kf, st:st + sz],
                        w2sb[:, kf, :512], kf == 0, kf == FFCH - 1)
                for kf in range(FFCH):
                    _mm(nc, psB[:sz], hT[:, kf, st:st + sz],
                        w2sb[:, kf, 512:], kf == 0, kf == FFCH - 1)
                oew = wc.tile([P, D], BF16, tag="oew")
                nc.vector.tensor_tensor(oew[:sz, :512], psA[:sz],
                                        b2_t[:sz, :512], ALU.add)
                nc.vector.tensor_tensor(oew[:sz, 512:], psB[:sz],
                                        b2_t[:sz, 512:], ALU.add)
                nc.vector.tensor_scalar_mul(oew[:sz], oew[:sz],
                                            wg_t[:sz, cb:cb + 1])
                nc.gpsimd.indirect_dma_start(
                    out=partial_d[:],
                    out_offset=bass.IndirectOffsetOnAxis(
                        ap=id_i[:sz, cb:cb + 1], axis=0),
                    in_=oew[:sz], in_offset=None)

            # 8-core ReduceScatter of expert contributions + residual
            nc.gpsimd.collective_compute(
                "ReduceScatter", ALU.add,
                replica_groups=[[0, 1, 2, 3, 4, 5, 6, 7]],
                ins=[partial_d[:T, :].opt()], outs=[moe_sh[:].opt()])
            moe_t = wc.tile([P, D], BF16, tag="moet")
            nc.sync.dma_start(moe_t[:], moe_sh[:])
            out_t = wc.tile([P, D], F32, tag="outt")
            nc.vector.tensor_tensor(out_t[:], moe_t[:], xs_t[:], ALU.add)
            nc.sync.dma_start(tn["out_sh"][:], out_t[:])


_CACHED = {}


def _get_nc():
    if "nc" not in _CACHED:
        nc = bacc.Bacc("TRN2", target_bir_lowering=False, debug=False,
                       num_devices=NCORES)
        build(nc)
        nc.compile()
        _CACHED["nc"] = nc
    return _CACHED["nc"]


def make_in_maps(inputs):
    bf16 = ml_dtypes.bfloat16
    x = np.asarray(inputs["x"], np.float32)
    rope_cos = np.asarray(inputs["rope_cos"], np.float32)
    rope_sin = np.asarray(inputs["rope_sin"], np.float32)
    wq = np.asarray(inputs["wq"], np.float32)
    bq = np.asarray(inputs["bq"], np.float32)
    wk = np.asarray(inputs["wk"], np.float32)
    bk = np.asarray(inputs["bk"], np.float32)
    wv = np.asarray(inputs["wv"], np.float32)
    bv = np.asarray(inputs["bv"], np.float32)
    wo = np.asarray(inputs["wo"], np.float32)
    bo = np.asarray(inputs["bo"], np.float32)
    n1w = np.asarray(inputs["norm1_w"], np.float32)
    n2w = np.asarray(inputs["norm2_w"], np.float32)
    rw = np.asarray(inputs["router_w"], np.float32)
    rb = np.asarray(inputs["router_b"], np.float32)
    w1 = np.asarray(inputs["w1"], np.float32)
    b1 = np.asarray(inputs["b1"], np.float32)
    w2 = np.asarray(inputs["w2"], np.float32)
    b2 = np.asarray(inputs["b2"], np.float32)

    xf = x.reshape(T, D)
    xpb_full = (xf + bo[None, :]).astype(np.float32)
    mtri = np.where(np.arange(P)[:, None] >= np.arange(P)[None, :], 0.0,
                    -1e5).astype(np.float32)
    tokid = (np.arange(P)[:, None] + P * np.arange(TCH)[None, :]).astype(
        np.float32)
    iotac = np.arange(C_CAP, dtype=np.float32)[None, :]
    rw_scaled = (rw * n2w[:, None]).astype(np.float32)
    iscl = 1.0 / np.sqrt(HD)  # score scale folded into wq/bq
    wqn = (wq * n1w[:, None] * iscl).astype(np.float32)
    wkn = (wk * n1w[:, None]).astype(np.float32)
    wvn = (wv * n1w[:, None]).astype(np.float32)
    # packed router weights: rw_packed[p, kd*E+e] = rw_scaled[kd*128+p, e]
    rw_packed = np.ascontiguousarray(
        rw_scaled.reshape(DCH, P, E).transpose(1, 0, 2).reshape(P, DCH * E))
    cos2T = np.ascontiguousarray(np.tile(rope_cos.T, (2, 1)).astype(bf16))
    sin2T = np.ascontiguousarray(np.tile(rope_sin.T, (2, 1)).astype(bf16))
    # rot_half as matmul: out[m] = sum_k rotm[k, m] * in[k] per 64-block
    r64 = np.zeros((HD, HD), np.float32)
    for m in range(HD // 2):
        r64[m + HD // 2, m] = -1.0
    for m in range(HD // 2, HD):
        r64[m - HD // 2, m] = 1.0
    rotm = np.zeros((P, P), bf16)
    rotm[:HD, :HD] = r64
    rotm[HD:, HD:] = r64
    # w1 pre-permuted (n2w folded in):
    # w1h[c][mf, p, kd*128+f] = n2w[kd*128+p] * w1[c][kd*128+p, mf*128+f]
    w1n = w1 * n2w[None, :, None]
    w1h = [np.ascontiguousarray(
        w1n[c].reshape(DCH, P, FFCH, P).transpose(2, 1, 0, 3).reshape(
            FFCH, P, D).astype(bf16)) for c in range(NCORES)]

    in_maps = []
    for c in range(NCORES):
        b, g = c // 4, c % 4
        esel = np.zeros((1, E), np.float32)
        esel[0, c] = 1.0
        in_maps.append({
            "xb": np.ascontiguousarray(x[b].astype(bf16)),
            "xpb": np.ascontiguousarray(xpb_full[c * P:(c + 1) * P]),
            "cosT": cos2T,
            "sinT": sin2T,
            "rotm": rotm,
            "wq": np.ascontiguousarray(
                wqn[:, g * 4 * HD:(g + 1) * 4 * HD].astype(bf16)),
            "wk": np.ascontiguousarray(np.tile(
                wkn[:, g * HD:(g + 1) * HD], (1, 2)).astype(bf16)),
            "wv": np.ascontiguousarray(
                wvn[:, g * HD:(g + 1) * HD].astype(bf16)),
            "bq": np.ascontiguousarray(
                (bq[g * 4 * HD:(g + 1) * 4 * HD] * iscl).reshape(2, P).T),
            "bk": np.ascontiguousarray(
                np.tile(bk[g * HD:(g + 1) * HD], 2)[:, None]),
            "bv": np.ascontiguousarray(bv[None, g * HD:(g + 1) * HD]),
            "wo": np.ascontiguousarray(
                wo[g * 4 * HD:(g + 1) * 4 * HD, :].astype(bf16)),
            "rw": rw_packed,
            "rb": np.ascontiguousarray(rb[None, :]),
            "xpbrw": np.ascontiguousarray(
                xpb_full[c * P:(c + 1) * P] @ rw_scaled),
            "mtri": mtri,
            "w1": w1h[c],
            "b1T": np.ascontiguousarray(b1[c].reshape(FFCH, P).T),
            "w2": np.ascontiguousarray(w2[c].astype(bf16)),
            "b2": np.ascontiguousarray(b2[c][None, :]),
            "tokid": tokid,
            "iotac": iotac,
            "esel": esel,
        })
    return in_maps


def kernel(**inputs) -> np.ndarray:
    in_maps = make_in_maps(inputs)
    nc = _get_nc()
    res = bass_utils.run_bass_kernel_spmd(nc, in_maps,
                                          core_ids=list(range(NCORES)))
    out = np.concatenate([res.results[c]["out_sh"] for c in range(NCORES)], 0)
    return out.reshape(B, S, D)
